# revision 1
# baseline (speedup 1.0000x reference)
"""Distributed Trainium2 Bass kernel for nn_Attention_27659589386447.

Reference computation (B=2, S=2048, D=1024, H=16, HD=64):
    xq = x @ Wq.T ; xk = x @ Wq.T (key uses query weights!) ; xv = x @ Wv.T
    q = rope(xq), k = rope(xk)  -> k == q
    out = causal_softmax(q @ k.T / sqrt(HD)) @ v     per (batch, head)

Sharding (8 cores): core c -> batch b = c // 4, head group g = c % 4
(heads 4g..4g+3, feature slice m = 256*g .. 256*(g+1)).
Each core's output slice is fully independent -> no collectives.

Device algorithm per core (all matmuls bf16, accumulation f32):
  - inputs arrive host-pre-transposed/cast: xT [1024,2048] bf16,
    WqT/WvT [1024,256] bf16, rope tables [128,2048] bf16 (see pack_* below)
  - qT = WqT.T @ xT  (per 128-row tile: 2 heads packed, RoPE'd on DVE)
  - v  = xT.T @ WvT  (natural layout, ones column appended per head)
  - scoresT[j,i] = k_j . q_i computed TRANSPOSED so exp(scoresT) tiles feed
    the PV matmul directly as the moving operand (no P transposes);
    softmax denominator = ones-column of V (out row 64); no max-subtraction
    (scores bounded, exp in f32 on ACT with fused 1/sqrt(HD) scale)
  - PV: outT[65, i] += v[j,:65].T @ exp(scoresT)[j, i]  accumulated in PSUM
  - PE-transpose outT back to natural [i, 64+1], scale by 1/denominator,
    DMA out f32.

RoPE rotate-half needs partition-shifted reads, which compute engines cannot
do; the shuffled copy of qT is produced with four SBUF->SBUF DMAs instead.
The sign of sin is folded into the host-prepared table.

Scheduling: both q projections and the first 4 v-tiles form the head block;
the remaining v-projection tiles and the output transpose-back/normalize
steps are drip-fed one-per-attention-tile into the PE slack of the
ACT(exp)-paced attention inner loop.
"""

import sys

if "/opt/trn_rl_repo" not in sys.path:
    sys.path.insert(0, "/opt/trn_rl_repo")

import numpy as np
import ml_dtypes

BF16 = ml_dtypes.bfloat16

B, S, D, H = 2, 2048, 1024, 16
HD = 64
N_CORES = 8
M = 256           # features per core (4 heads)
NK = D // 128     # 8 contraction chunks
NST = S // 128    # 16 s-tiles
NSC = S // 512    # 4 s-chunks


# --------------------------------------------------------------------------
# host-side packing
# --------------------------------------------------------------------------

_PERM = np.arange(128)  # natural layout: head h at partitions 64h..64h+64


def pack_inputs(x, Wq, Wv, cos, sin):
    """Builds the 8 per-core input maps (host-side shard + layout prep)."""
    xt_b = []
    for b in range(B):
        xt_b.append(np.ascontiguousarray(x[b].T).astype(BF16))  # [1024, 2048]

    cosT = np.ascontiguousarray(cos.T).astype(np.float32)  # [64, 2048]
    sinT = np.ascontiguousarray(sin.T).astype(np.float32)
    # signed sin: s'[d] = -sin[d] (d<32), +sin[d] (d>=32)
    sinS = np.concatenate([-sinT[:32], sinT[32:]], axis=0)  # [64, 2048]
    # per-tile permuted rope rows (same permutation for both head halves)
    d_of_p = (_PERM % 64)
    cosd = cosT[d_of_p].astype(BF16)          # [128, 2048]
    sind = sinS[d_of_p].astype(BF16)          # [128, 2048]

    in_maps = []
    for c in range(N_CORES):
        b, g = c // 4, c % 4
        mr = slice(g * M, (g + 1) * M)
        wqt = np.ascontiguousarray(Wq[mr].T).astype(BF16)  # [1024, 256]
        wvt = np.ascontiguousarray(Wv[mr].T).astype(BF16)
        in_maps.append({
            "xt": xt_b[b],
            "wqt": wqt,
            "wvt": wvt,
            "cosd": cosd,
            "sind": sind,
        })
    return in_maps


def gather_outputs(results):
    out = np.empty((B, S, D), dtype=np.float32)
    for c in range(N_CORES):
        b, g = c // 4, c % 4
        out[b, :, g * M:(g + 1) * M] = results[c]["out"]
    return out


# --------------------------------------------------------------------------
# device graph
# --------------------------------------------------------------------------

def build_graph(num_devices=N_CORES, repeat=1):
    from concourse import bacc, tile, mybir
    from concourse.masks import make_upper_triangular, make_identity

    bf16 = mybir.dt.bfloat16
    f32 = mybir.dt.float32

    nc = bacc.Bacc("TRN2", target_bir_lowering=False, debug=False,
                   num_devices=num_devices)

    xt_e = nc.dram_tensor("xt", [D, S], bf16, kind="ExternalInput")
    wqt_e = nc.dram_tensor("wqt", [D, M], bf16, kind="ExternalInput")
    wvt_e = nc.dram_tensor("wvt", [D, M], bf16, kind="ExternalInput")
    cosd_e = nc.dram_tensor("cosd", [128, S], bf16, kind="ExternalInput")
    sind_e = nc.dram_tensor("sind", [128, S], bf16, kind="ExternalInput")
    out_e = nc.dram_tensor("out", [S, M], f32, kind="ExternalOutput")

    with tile.TileContext(nc) as tc:
        with (
            tc.tile_pool(name="persist", bufs=1) as pp,
            tc.tile_pool(name="work", bufs=4) as wp,
            tc.tile_pool(name="rope", bufs=3) as rp,
            tc.tile_pool(name="ps2", bufs=2, space="PSUM") as ps2,
            tc.tile_pool(name="pspv", bufs=2, space="PSUM") as ppv,
            tc.tile_pool(name="pstb", bufs=1, space="PSUM") as ptb,
        ):
            xts = [pp.tile([128, S], bf16, tag=f"xt{k}", name=f"xt{k}")
                   for k in range(NK)]
            wqts = [pp.tile([128, M], bf16, tag=f"wq{k}", name=f"wq{k}")
                    for k in range(NK)]
            wvts = [pp.tile([128, M], bf16, tag=f"wv{k}", name=f"wv{k}")
                    for k in range(NK)]
            cosd = pp.tile([128, S], bf16, tag="cosd", name="cosd")
            sind = pp.tile([128, S], bf16, tag="sind", name="sind")
            q_sb = [pp.tile([128, S], bf16, tag=f"q{mt}", name=f"q{mt}")
                    for mt in range(2)]
            v_sb = [pp.tile([128, 4 * 65], bf16, tag=f"v{st}", name=f"v{st}")
                    for st in range(NST)]
            out_sb = [pp.tile([128, M], f32, tag=f"o{st}", name=f"o{st}")
                      for st in range(NST)]
            mask = pp.tile([128, 128], bf16, tag="mask", name="mask")
            ident = pp.tile([65, 65], bf16, tag="ident", name="ident")

            # ---- constants
            make_upper_triangular(nc, mask[:, :], val=1.0, diag=True)
            make_identity(nc, ident[:, :])

            def load_inputs():
                # interleave weight/x loads and split across both HWDGE
                # queues (SP + ACT) so the projection k-chains start early
                for k in range(NK):
                    eng = nc.sync if k % 2 == 0 else nc.scalar
                    eng.dma_start(out=wqts[k],
                                  in_=wqt_e[128 * k:128 * (k + 1), :])
                    eng.dma_start(out=xts[k],
                                  in_=xt_e[128 * k:128 * (k + 1), :])
                nc.sync.dma_start(out=cosd, in_=cosd_e[:, :])
                nc.scalar.dma_start(out=sind, in_=sind_e[:, :])
                for k in range(NK):
                    eng = nc.sync if k % 2 == 0 else nc.scalar
                    eng.dma_start(out=wvts[k],
                                  in_=wvt_e[128 * k:128 * (k + 1), :])

            def proj_q(mt):
                # two [128,1024] psum tiles hold the four 512-wide s-chunks
                pstiles = [ps2.tile([128, 1024], f32, tag="ps", name="ps")
                           for _ in range(2)]
                for k in range(NK):
                    for sc in range(NSC):
                        ps = pstiles[sc // 2][:, 512 * (sc % 2):
                                              512 * (sc % 2) + 512]
                        nc.tensor.matmul(
                            ps,
                            wqts[k][:, 128 * mt:128 * (mt + 1)],
                            xts[k][:, 512 * sc:512 * (sc + 1)],
                            start=(k == 0), stop=(k == NK - 1),
                        )
                qraw = rp.tile([128, S], bf16, tag="qraw", name="qraw",
                               bufs=2)
                qshuf = rp.tile([128, S], bf16, tag="qshuf", name="qshuf",
                                bufs=2)
                for half in range(2):
                    hsl = slice(1024 * half, 1024 * (half + 1))
                    nc.scalar.copy(out=qraw[:, hsl], in_=pstiles[half][:, :])
                    # rotate-half: swap 32-row halves of each head via DMA
                    for h in range(2):
                        p = 64 * h
                        nc.sync.dma_start(out=qshuf[p:p + 32, hsl],
                                          in_=qraw[p + 32:p + 64, hsl])
                        nc.sync.dma_start(out=qshuf[p + 32:p + 64, hsl],
                                          in_=qraw[p:p + 32, hsl])
                    for sc in (2 * half, 2 * half + 1):
                        ssl = slice(512 * sc, 512 * (sc + 1))
                        tmp = rp.tile([128, 512], bf16, tag="rtmp",
                                      name="rtmp")
                        nc.vector.tensor_mul(tmp[:, :], qshuf[:, ssl],
                                             sind[:, ssl])
                        tmp2 = rp.tile([128, 512], bf16, tag="rtmp2",
                                       name="rtmp2")
                        nc.vector.tensor_mul(tmp2[:, :], qraw[:, ssl],
                                             cosd[:, ssl])
                        nc.vector.tensor_add(q_sb[mt][:, ssl], tmp2[:, :],
                                             tmp[:, :])

            def proj_v_steps(st):
                """Yield filler callables computing v_sb[st] (1-bank psum)."""
                psv = ptb.tile([128, 512], f32, tag="aux", name="vps",
                               bufs=2)

                def mms(k0, psv=psv, st=st):
                    for k in range(k0, k0 + 2):
                        nc.tensor.matmul(
                            psv[:, 0:M],
                            xts[k][:, 128 * st:128 * (st + 1)],
                            wvts[k][:, :],
                            start=(k == 0), stop=(k == NK - 1),
                        )

                def copies(psv=psv, st=st):
                    nc.gpsimd.memset(v_sb[st][:, :], 1.0)
                    eng = nc.scalar if st < 4 else nc.vector
                    copy = (eng.copy if st < 4 else eng.tensor_copy)
                    for h in range(4):
                        copy(
                            out=v_sb[st][:, 65 * h:65 * h + 64],
                            in_=psv[:, 64 * h:64 * (h + 1)],
                        )

                for k0 in range(0, NK, 2):
                    yield lambda k0=k0, f=mms: f(k0)
                yield copies

            def proj_v(st_lo, st_hi):
                for st in range(st_lo, st_hi):
                    for step in proj_v_steps(st):
                        step()

            def emit_tb(h, c, q4, otsb):
                """PE-transpose one 128-col block of otsb back to natural
                layout and write the normalized slice of out_sb."""
                st = 4 * c + q4
                tb = ptb.tile([128, 512], bf16, tag="aux", name="tb",
                              bufs=2)
                nc.tensor.transpose(
                    tb[:, 0:65],
                    otsb[:, 128 * q4:128 * (q4 + 1)],
                    ident[:, :],
                )
                rec = wp.tile([128, 1], f32, tag="rec", name="rec", bufs=8)
                nc.vector.reciprocal(out=rec[:, :], in_=tb[:, 64:65])
                nc.vector.tensor_scalar_mul(
                    out_sb[st][:, 64 * h:64 * (h + 1)],
                    tb[:, 0:64],
                    rec[:, :],
                )

            def attention(hp, c, pending, fillers=None, rate=1):
                """pending: deferred emit_tb args; fillers: queue of callables
                (remaining v-proj steps) popped `rate` per J-tile."""
                qt = q_sb[hp]
                pvA = ppv.tile([65, 512], f32, tag="pv", name="pv")
                pvB = ppv.tile([65, 512], f32, tag="pv", name="pv")
                njt = 4 * c + 4
                for J in range(njt):
                    off = 0 if J <= 4 * c else 128 * (J - 4 * c)
                    n = 512 - off
                    g0 = 512 * c + off
                    jsl = slice(128 * J, 128 * (J + 1))
                    isl = slice(g0, g0 + n)
                    psqk = ps2.tile([128, 1024], f32, tag="ps", name="ps")
                    for a, (p0, p1) in enumerate(((0, 64), (64, 128))):
                        nc.tensor.matmul(
                            psqk[:, 512 * a:512 * a + n],
                            qt[p0:p1, jsl],
                            qt[p0:p1, isl],
                            start=True, stop=True,
                            tile_position=(p0, 0),
                        )
                    pt = wp.tile([128, 1024], bf16, tag="pt", name="pt")
                    if n == 512:
                        nc.scalar.activation(
                            out=pt[:, :], in_=psqk[:, :],
                            func=mybir.ActivationFunctionType.Exp,
                            scale=0.125,
                        )
                    else:
                        for a in range(2):
                            nc.scalar.activation(
                                out=pt[:, 512 * a:512 * a + n],
                                in_=psqk[:, 512 * a:512 * a + n],
                                func=mybir.ActivationFunctionType.Exp,
                                scale=0.125,
                            )
                    if J >= 4 * c:
                        for a in range(2):
                            nc.gpsimd.tensor_mul(
                                pt[:, 512 * a:512 * a + 128],
                                pt[:, 512 * a:512 * a + 128],
                                mask[:, :])
                    for a, pv in enumerate((pvA, pvB)):
                        h = 2 * hp + a
                        nc.tensor.matmul(
                            pv[:, off:off + n],
                            v_sb[J][:, 65 * h:65 * h + 65],
                            pt[:, 512 * a:512 * a + n],
                            start=(J == 0), stop=(J == njt - 1),
                            skip_group_check=True,
                        )
                    popped = 0
                    while fillers and popped < rate:
                        fillers.pop(0)()
                        popped += 1
                    if pending:
                        emit_tb(*pending.pop(0))
                # free the pv psums now; defer the PE transposes
                for a, pv in enumerate((pvA, pvB)):
                    h = 2 * hp + a
                    otsb = wp.tile([65, 512], bf16, tag="otsb", name="otsb",
                                   bufs=6)
                    nc.vector.tensor_copy(out=otsb[:, :], in_=pv[:, :])
                    for q4 in range(4):
                        pending.append((h, c, q4, otsb))

            for _rep in range(repeat):
                load_inputs()
                proj_q(0)
                proj_q(1)
                proj_v(0, 4)
                fillers = []
                for st in range(4, NST):
                    fillers.extend(proj_v_steps(st))
                pending = []
                rates = [3, 2, 2, 1]
                for c in range(NSC):
                    attention(0, c, pending, fillers, rates[c])
                for c in range(NSC):
                    attention(1, c, pending, fillers, 1)
                for f in fillers:
                    f()
                for args in pending:
                    emit_tb(*args)
                for st in range(NST):
                    nc.sync.dma_start(out=out_e[128 * st:128 * (st + 1), :],
                                      in_=out_sb[st])

    nc.compile()
    return nc


_NC = None


def get_graph():
    global _NC
    if _NC is None:
        _NC = build_graph()
    return _NC


# --------------------------------------------------------------------------
# execution (PJRT via axon), cached jitted runner
# --------------------------------------------------------------------------

_RUNNER = None


class _Runner:
    """Builds the sharded jit once; callable with a list of 8 in_maps."""

    def __init__(self, nc):
        import jax
        import numpy as _np
        from jax.sharding import Mesh, PartitionSpec
        from jax.experimental.shard_map import shard_map
        from concourse import bass2jax, mybir
        from concourse.bass2jax import (_bass_exec_p, install_neuronx_cc_hook,
                                        partition_id_tensor)

        install_neuronx_cc_hook()
        self.jax = jax
        self.nc = nc
        partition_name = (nc.partition_id_tensor.name
                          if nc.partition_id_tensor else None)

        in_names = []
        out_names = []
        out_avals = []
        zero_shapes = []
        for alloc in nc.m.functions[0].allocations:
            if not isinstance(alloc, mybir.MemoryLocationSet):
                continue
            name = alloc.memorylocations[0].name
            if alloc.kind == "ExternalInput":
                if name != partition_name:
                    in_names.append(name)
            elif alloc.kind == "ExternalOutput":
                shape = tuple(alloc.tensor_shape)
                dtype = mybir.dt.np(alloc.dtype)
                out_names.append(name)
                out_avals.append(jax.core.ShapedArray(shape, dtype))
                zero_shapes.append((shape, dtype))
        self.in_names = list(in_names)
        self.out_names = out_names
        self.out_avals = out_avals
        self.zero_shapes = zero_shapes
        n_params = len(in_names)
        n_outs = len(out_names)
        all_in_names = in_names + out_names
        if partition_name is not None:
            all_in_names = all_in_names + [partition_name]
        self.all_in_names = all_in_names
        self.partition_name = partition_name

        def _body(*args):
            operands = list(args)
            if partition_name is not None:
                operands.append(partition_id_tensor())
            outs = _bass_exec_p.bind(
                *operands,
                out_avals=tuple(out_avals),
                in_names=tuple(all_in_names),
                out_names=tuple(out_names),
                lowering_input_output_aliases=(),
                sim_require_finite=True,
                sim_require_nnan=True,
                nc=nc,
            )
            return tuple(outs)

        devices = jax.devices()[:N_CORES]
        mesh = Mesh(np.asarray(devices), ("core",))
        self.mesh = mesh
        in_specs = (PartitionSpec("core"),) * (n_params + n_outs)
        out_specs = (PartitionSpec("core"),) * n_outs
        donate = tuple(range(n_params, n_params + n_outs))
        self.sharded = jax.jit(
            shard_map(_body, mesh=mesh, in_specs=in_specs,
                      out_specs=out_specs, check_rep=False),
            donate_argnums=donate, keep_unused=True,
        )

    def make_looped(self, n_iters):
        """Jitted fn running the NEFF n_iters times on-device (for timing)."""
        import jax
        import jax.numpy as jnp
        from jax.sharding import PartitionSpec
        from jax.experimental.shard_map import shard_map
        from concourse.bass2jax import _bass_exec_p, partition_id_tensor

        nc = self.nc
        out_avals = self.out_avals
        all_in_names = self.all_in_names
        out_names = self.out_names
        partition_name = self.partition_name

        def _loop(*args):
            def body(i, acc):
                operands = list(args)
                if partition_name is not None:
                    operands.append(partition_id_tensor())
                outs = _bass_exec_p.bind(
                    *operands,
                    out_avals=tuple(out_avals),
                    in_names=tuple(all_in_names),
                    out_names=tuple(out_names),
                    lowering_input_output_aliases=(),
                    sim_require_finite=True,
                    sim_require_nnan=True,
                    nc=nc,
                )
                return acc + outs[0][0, 0].astype(jnp.float32)

            acc = jax.lax.fori_loop(0, n_iters, body, jnp.float32(0.0))
            return (acc.reshape(1),)

        n_in = len(self.in_names) + len(self.out_names)
        return jax.jit(shard_map(
            _loop, mesh=self.mesh,
            in_specs=(PartitionSpec("core"),) * n_in,
            out_specs=(PartitionSpec("core"),),
            check_rep=False,
        ))

    def concat_inputs(self, in_maps):
        return [
            np.concatenate([np.asarray(in_maps[c][n]) for c in range(N_CORES)],
                           axis=0)
            for n in self.in_names
        ]

    def make_zeros(self):
        return [np.zeros((N_CORES * s[0], *s[1:]), d)
                for (s, d) in self.zero_shapes]

    def __call__(self, in_maps):
        concat_in = self.concat_inputs(in_maps)
        out_arrs = self.sharded(*concat_in, *self.make_zeros())
        return [
            {name: np.asarray(out_arrs[i]).reshape(
                N_CORES, *self.out_avals[i].shape)[c]
             for i, name in enumerate(self.out_names)}
            for c in range(N_CORES)
        ]


def get_runner():
    global _RUNNER
    if _RUNNER is None:
        _RUNNER = _Runner(get_graph())
    return _RUNNER


def kernel(x, Wq, Wv, cos, sin):
    x = np.asarray(x, dtype=np.float32)
    Wq = np.asarray(Wq, dtype=np.float32)
    Wv = np.asarray(Wv, dtype=np.float32)
    cos = np.asarray(cos, dtype=np.float32)
    sin = np.asarray(sin, dtype=np.float32)
    in_maps = pack_inputs(x, Wq, Wv, cos, sin)
    results = get_runner()(in_maps)
    return gather_outputs(results)



# revision 24
# speedup vs baseline: 3.4520x; 3.4520x over previous
"""Distributed Trainium2 Bass kernel for nn_Attention_27659589386447.

Reference computation (B=2, S=2048, D=1024, H=16, HD=64):
    xq = x @ Wq.T ; xk = x @ Wq.T (key uses query weights!) ; xv = x @ Wv.T
    q = rope(xq), k = rope(xk)  -> k == q
    out = causal_softmax(q @ k.T / sqrt(HD)) @ v     per (batch, head)

Sharding (8 cores): core c -> batch b = c // 4, head group g = c % 4
(heads 4g..4g+3, feature slice m = 256*g .. 256*(g+1)).
Each core's output slice is fully independent -> no collectives.

Device algorithm per core (all matmuls bf16, accumulation f32):
  - inputs arrive host-pre-transposed/cast: xT [1024,2048] bf16,
    WqT/WvT [1024,256] bf16, rope tables [128,2048] bf16 (see pack_* below)
  - qT = WqT.T @ xT  (per 128-row tile: 2 heads packed, RoPE'd on DVE)
  - v  = xT.T @ WvT  (natural layout, ones column appended per head)
  - scoresT[j,i] = k_j . q_i computed TRANSPOSED so exp(scoresT) tiles feed
    the PV matmul directly as the moving operand (no P transposes);
    softmax denominator = ones-column of V (out row 64); no max-subtraction
    (scores bounded, exp in f32 on ACT with fused 1/sqrt(HD) scale)
  - PV: outT[65, i] += v[j,:65].T @ exp(scoresT)[j, i]  accumulated in PSUM
  - PE-transpose outT back to natural [i, 64+1], scale by 1/denominator,
    DMA out f32.

RoPE rotate-half needs partition-shifted reads, which compute engines cannot
do; the shuffled copy of qT is produced with four SBUF->SBUF DMAs instead.
The sign of sin is folded into the host-prepared table.

Scheduling: both q projections and the first 4 v-tiles form the head block;
the remaining v-projection tiles and the output transpose-back/normalize
steps are drip-fed one-per-attention-tile into the PE slack of the
ACT(exp)-paced attention inner loop.
"""

import sys

if "/opt/trn_rl_repo" not in sys.path:
    sys.path.insert(0, "/opt/trn_rl_repo")

import numpy as np
import ml_dtypes

BF16 = ml_dtypes.bfloat16

B, S, D, H = 2, 2048, 1024, 16
HD = 64
N_CORES = 8
M = 256           # features per core (4 heads)
NK = D // 128     # 8 contraction chunks
NST = S // 128    # 16 s-tiles
NSC = S // 512    # 4 s-chunks


# --------------------------------------------------------------------------
# host-side packing
# --------------------------------------------------------------------------

_PERM = np.arange(128)  # natural layout: head h at partitions 64h..64h+64


def pack_inputs(x, Wq, Wv, cos, sin):
    """Builds the 8 per-core input maps (host-side shard + layout prep)."""
    xt_b = []
    for b in range(B):
        xt_b.append(np.ascontiguousarray(x[b].T).astype(BF16))  # [1024, 2048]

    cosT = np.ascontiguousarray(cos.T).astype(np.float32)  # [64, 2048]
    sinT = np.ascontiguousarray(sin.T).astype(np.float32)
    # signed sin: s'[d] = -sin[d] (d<32), +sin[d] (d>=32)
    sinS = np.concatenate([-sinT[:32], sinT[32:]], axis=0)  # [64, 2048]
    # per-tile permuted rope rows (same permutation for both head halves)
    d_of_p = (_PERM % 64)
    cosd = cosT[d_of_p].astype(BF16)          # [128, 2048]
    sind = sinS[d_of_p].astype(BF16)          # [128, 2048]

    in_maps = []
    for c in range(N_CORES):
        b, g = c // 4, c % 4
        mr = slice(g * M, (g + 1) * M)
        wqt = np.ascontiguousarray(Wq[mr].T).astype(BF16)  # [1024, 256]
        wvt = np.ascontiguousarray(Wv[mr].T).astype(BF16)
        in_maps.append({
            "xt": xt_b[b],
            "wqt": wqt,
            "wvt": wvt,
            "cosd": cosd,
            "sind": sind,
        })
    return in_maps


def gather_outputs(results):
    out = np.empty((B, S, D), dtype=np.float32)
    for c in range(N_CORES):
        b, g = c // 4, c % 4
        out[b, :, g * M:(g + 1) * M] = results[c]["out"]
    return out


# --------------------------------------------------------------------------
# device graph
# --------------------------------------------------------------------------

def build_graph(num_devices=N_CORES, repeat=1):
    from concourse import bacc, tile, mybir
    from concourse.masks import make_upper_triangular, make_identity

    bf16 = mybir.dt.bfloat16
    f32 = mybir.dt.float32

    nc = bacc.Bacc("TRN2", target_bir_lowering=False, debug=False,
                   num_devices=num_devices)

    xt_e = nc.dram_tensor("xt", [D, S], bf16, kind="ExternalInput")
    wqt_e = nc.dram_tensor("wqt", [D, M], bf16, kind="ExternalInput")
    wvt_e = nc.dram_tensor("wvt", [D, M], bf16, kind="ExternalInput")
    cosd_e = nc.dram_tensor("cosd", [128, S], bf16, kind="ExternalInput")
    sind_e = nc.dram_tensor("sind", [128, S], bf16, kind="ExternalInput")
    out_e = nc.dram_tensor("out", [S, M], f32, kind="ExternalOutput")

    with tile.TileContext(nc) as tc:
        with (
            tc.tile_pool(name="persist", bufs=1) as pp,
            tc.tile_pool(name="inload", bufs=2) as ip,
            tc.tile_pool(name="work", bufs=4) as wp,
            tc.tile_pool(name="rope", bufs=3) as rp,
            tc.tile_pool(name="ps2", bufs=2, space="PSUM") as ps2,
            tc.tile_pool(name="pspv", bufs=2, space="PSUM") as ppv,
            tc.tile_pool(name="pstb", bufs=1, space="PSUM") as ptb,
        ):
            # input tiles are double-buffered (bufs=2) so the next repeat's
            # HBM loads overlap this repeat's attention phase
            def alloc_inputs():
                xts = [ip.tile([128, S], bf16, tag=f"xt{k}", name=f"xt{k}")
                       for k in range(NK)]
                wqts = [ip.tile([128, M], bf16, tag=f"wq{k}", name=f"wq{k}")
                        for k in range(NK)]
                wvts = [ip.tile([128, M], bf16, tag=f"wv{k}", name=f"wv{k}")
                        for k in range(NK)]
                cosd = ip.tile([128, S], bf16, tag="cosd", name="cosd")
                sind = ip.tile([128, S], bf16, tag="sind", name="sind")
                return xts, wqts, wvts, cosd, sind

            q_sb = [pp.tile([128, S], bf16, tag=f"q{mt}", name=f"q{mt}")
                    for mt in range(2)]
            v_sb = [pp.tile([128, 4 * 65], bf16, tag=f"v{st}", name=f"v{st}")
                    for st in range(NST)]
            out_sb = [pp.tile([128, M], f32, tag=f"o{st}", name=f"o{st}")
                      for st in range(NST)]
            masknegT = pp.tile([128, 128], bf16, tag="mask", name="mask")
            ident = pp.tile([65, 65], bf16, tag="ident", name="ident")
            idn128 = pp.tile([128, 128], bf16, tag="idn128", name="idn128")

            # ---- constants
            # additive causal mask, applied on the PE via identity moving:
            # psqk[j, x] += masknegT[x, j]; kill j > x with -240
            # (exp(0.125 * -240) ~ 1e-13), so masknegT is strictly upper
            make_upper_triangular(nc, masknegT[:, :], val=-240.0, diag=False)
            make_identity(nc, ident[:, :])
            make_identity(nc, idn128[:, :])

            def load_inputs():
                # interleave weight/x loads across the SP and Pool queues.
                # Both engines finish their per-repeat work early (SP: rope
                # shuffles; Pool: memsets), so the NEXT repeat's loads issue
                # mid-attention and hide completely.  Output DMAs go on the
                # ACT queue instead (see flush_tb).
                engs = [nc.sync, nc.gpsimd]
                for k in range(NK):
                    eng = engs[k % 2]
                    eng.dma_start(out=wqts[k],
                                  in_=wqt_e[128 * k:128 * (k + 1), :])
                    eng.dma_start(out=xts[k],
                                  in_=xt_e[128 * k:128 * (k + 1), :])
                nc.sync.dma_start(out=cosd, in_=cosd_e[:, :])
                nc.gpsimd.dma_start(out=sind, in_=sind_e[:, :])
                for k in range(NK):
                    eng = engs[k % 2]
                    eng.dma_start(out=wvts[k],
                                  in_=wvt_e[128 * k:128 * (k + 1), :])

            def proj_q(mt):
                # two [128,1024] psum tiles hold the four 512-wide s-chunks
                pstiles = [ps2.tile([128, 1024], f32, tag="ps", name="ps")
                           for _ in range(2)]
                for k in range(NK):
                    for sc in range(NSC):
                        ps = pstiles[sc // 2][:, 512 * (sc % 2):
                                              512 * (sc % 2) + 512]
                        nc.tensor.matmul(
                            ps,
                            wqts[k][:, 128 * mt:128 * (mt + 1)],
                            xts[k][:, 512 * sc:512 * (sc + 1)],
                            start=(k == 0), stop=(k == NK - 1),
                        )
                qraw = rp.tile([128, S], bf16, tag="qraw", name="qraw",
                               bufs=2)
                qshuf = rp.tile([128, S], bf16, tag="qshuf", name="qshuf",
                                bufs=2)
                for half in range(2):
                    hsl = slice(1024 * half, 1024 * (half + 1))
                    # DVE, not ACT: ACT paces the attention phase
                    nc.vector.tensor_copy(out=qraw[:, hsl],
                                          in_=pstiles[half][:, :])
                    # rotate-half: swap 32-row halves of each head via DMA
                    for h in range(2):
                        p = 64 * h
                        nc.sync.dma_start(out=qshuf[p:p + 32, hsl],
                                          in_=qraw[p + 32:p + 64, hsl])
                        nc.sync.dma_start(out=qshuf[p + 32:p + 64, hsl],
                                          in_=qraw[p:p + 32, hsl])
                    for sc in (2 * half, 2 * half + 1):
                        ssl = slice(512 * sc, 512 * (sc + 1))
                        tmp = rp.tile([128, 512], bf16, tag="rtmp",
                                      name="rtmp")
                        nc.vector.tensor_mul(tmp[:, :], qshuf[:, ssl],
                                             sind[:, ssl])
                        tmp2 = rp.tile([128, 512], bf16, tag="rtmp2",
                                       name="rtmp2")
                        nc.vector.tensor_mul(tmp2[:, :], qraw[:, ssl],
                                             cosd[:, ssl])
                        nc.vector.tensor_add(q_sb[mt][:, ssl], tmp2[:, :],
                                             tmp[:, :])

            def proj_v_steps(st):
                """Yield filler callables computing v_sb[st] (1-bank psum)."""
                psv = ptb.tile([128, 512], f32, tag="aux", name="vps",
                               bufs=2)

                def mms(k0, psv=psv, st=st):
                    for k in range(k0, k0 + 2):
                        nc.tensor.matmul(
                            psv[:, 0:M],
                            xts[k][:, 128 * st:128 * (st + 1)],
                            wvts[k][:, :],
                            start=(k == 0), stop=(k == NK - 1),
                        )

                def copies(psv=psv, st=st):
                    nc.gpsimd.memset(v_sb[st][:, :], 1.0)
                    for h in range(4):
                        nc.vector.tensor_copy(
                            out=v_sb[st][:, 65 * h:65 * h + 64],
                            in_=psv[:, 64 * h:64 * (h + 1)],
                        )

                for k0 in range(0, NK, 2):
                    yield lambda k0=k0, f=mms: f(k0)
                yield copies

            def proj_v(st_lo, st_hi):
                for st in range(st_lo, st_hi):
                    for step in proj_v_steps(st):
                        step()

            def emit_tb(h, c, q4, otsb):
                """PE-transpose one 128-col block of otsb back to natural
                layout and write the normalized slice of out_sb."""
                st = 4 * c + q4
                tb = ptb.tile([128, 512], bf16, tag="aux", name="tb",
                              bufs=2)
                nc.tensor.transpose(
                    tb[:, 0:65],
                    otsb[:, 128 * q4:128 * (q4 + 1)],
                    ident[:, :],
                )
                rec = wp.tile([128, 1], f32, tag="rec", name="rec", bufs=8)
                nc.vector.reciprocal(out=rec[:, :], in_=tb[:, 64:65])
                nc.vector.tensor_scalar_mul(
                    out_sb[st][:, 64 * h:64 * (h + 1)],
                    tb[:, 0:64],
                    rec[:, :],
                )

            def make_stages(hp, c, pending):
                """Build the per-J pipeline stages for chunk (hp, c).

                Each stage is (qk_exp, pv): qk_exp emits the QK matmuls and
                the exp+mask; pv emits the PV accumulation (and, for the last
                J, the pv->otsb drain).  The driver runs pv one stage behind
                qk_exp so the PE never waits on the ACT exp.
                """
                qt = q_sb[hp]
                state = {}
                njt = 4 * c + 4
                stages = []
                for J in range(njt):
                    off = 0 if J <= 4 * c else 128 * (J - 4 * c)
                    n = 512 - off
                    g0 = 512 * c + off
                    jsl = slice(128 * J, 128 * (J + 1))
                    isl = slice(g0, g0 + n)

                    def qk_exp(J=J, off=off, n=n, jsl=jsl, isl=isl):
                        diag = J >= 4 * c
                        psqk = ps2.tile([128, 1024], f32, tag="ps",
                                        name="ps")
                        for a, (p0, p1) in enumerate(((0, 64), (64, 128))):
                            nc.tensor.matmul(
                                psqk[:, 512 * a:512 * a + n],
                                qt[p0:p1, jsl],
                                qt[p0:p1, isl],
                                start=True, stop=not diag,
                                tile_position=(p0, 0),
                                skip_group_check=True,
                            )
                        if diag:
                            # causal mask folded into the psum on the PE
                            # itself (no cross-engine latency in the
                            # QK->exp->PV chain): psqk[j, x] += maskneg[x, j]
                            # via identity moving operand
                            for a in range(2):
                                nc.tensor.matmul(
                                    psqk[:, 512 * a:512 * a + 128],
                                    masknegT[:, :],
                                    idn128[:, :],
                                    start=False, stop=True,
                                    skip_group_check=True,
                                )
                        pt = wp.tile([128, 1024], bf16, tag="pt", name="pt")
                        if n == 512:
                            nc.scalar.activation(
                                out=pt[:, :], in_=psqk[:, :],
                                func=mybir.ActivationFunctionType.Exp,
                                scale=0.125,
                            )
                        else:
                            # one instruction over both heads' partial
                            # columns via a strided 3D access pattern
                            src = psqk[:, :].rearrange(
                                "p (a x) -> p a x", a=2)[:, :, 0:n]
                            dst = pt[:, :].rearrange(
                                "p (a x) -> p a x", a=2)[:, :, 0:n]
                            nc.scalar.activation(
                                out=dst, in_=src,
                                func=mybir.ActivationFunctionType.Exp,
                                scale=0.125,
                            )
                        state[J] = pt

                    def pv_step(J=J, off=off, n=n):
                        if J == 0:
                            state["pvA"] = ppv.tile([65, 512], f32,
                                                    tag="pv", name="pv")
                            state["pvB"] = ppv.tile([65, 512], f32,
                                                    tag="pv", name="pv")
                        pt = state.pop(J)
                        for a, key in enumerate(("pvA", "pvB")):
                            h = 2 * hp + a
                            nc.tensor.matmul(
                                state[key][:, off:off + n],
                                v_sb[J][:, 65 * h:65 * h + 65],
                                pt[:, 512 * a:512 * a + n],
                                start=(J == 0), stop=(J == njt - 1),
                                skip_group_check=True,
                            )
                        if J == njt - 1:
                            # free the pv psums; defer the PE transposes
                            for a, key in enumerate(("pvA", "pvB")):
                                h = 2 * hp + a
                                otsb = wp.tile([65, 512], bf16, tag="otsb",
                                               name="otsb", bufs=6)
                                nc.vector.tensor_copy(out=otsb[:, :],
                                                      in_=state[key][:, :])
                                for q4 in range(4):
                                    pending.append((h, c, q4, otsb))

                    stages.append((qk_exp, pv_step))
                return stages

            def run_attention(chunks, pending, fillers):
                """chunks: list of (hp, c, rate). Runs all stages with the
                pv leg skewed one stage behind qk+exp, dripping fillers and
                deferred transposes into the PE slack after each pv."""
                stages = []
                for hp, c, rate in chunks:
                    for qk_exp, pv in make_stages(hp, c, pending):
                        stages.append((qk_exp, pv, rate))
                skew = 2  # pv lags qk+exp by two stages
                lagged = []
                for qk_exp, pv, rate in stages + [(None, None, 1)] * skew:
                    if qk_exp is not None:
                        qk_exp()
                    lagged.append(pv)
                    if len(lagged) > skew:
                        lagged.pop(0)()
                        popped = 0
                        while fillers and popped < rate:
                            fillers.pop(0)()
                            popped += 1
                        if pending:
                            flush_tb(pending.pop(0))

            def flush_tb(args):
                emit_tb(*args)
                h, c, q4, _ = args
                if h == 3:
                    # all four head-slices of out_sb[st] are now written
                    st = 4 * c + q4
                    nc.scalar.dma_start(out=out_e[128 * st:128 * (st + 1), :],
                                        in_=out_sb[st])

            for _rep in range(repeat):
                xts, wqts, wvts, cosd, sind = alloc_inputs()
                load_inputs()
                proj_q(0)
                proj_q(1)
                proj_v(0, 4)
                fillers = []
                for st in range(4, NST):
                    fillers.extend(proj_v_steps(st))
                pending = []
                chunks = [(0, c, r) for c, r in enumerate([3, 2, 2, 1])]
                chunks += [(1, c, 1) for c in range(NSC)]
                run_attention(chunks, pending, fillers)
                for f in fillers:
                    f()
                for args in pending:
                    flush_tb(args)

    nc.compile()
    return nc


_NC = None


def get_graph():
    global _NC
    if _NC is None:
        _NC = build_graph()
    return _NC


# --------------------------------------------------------------------------
# execution (PJRT via axon), cached jitted runner
# --------------------------------------------------------------------------

_RUNNER = None


class _Runner:
    """Builds the sharded jit once; callable with a list of 8 in_maps."""

    def __init__(self, nc):
        import jax
        import numpy as _np
        from jax.sharding import Mesh, PartitionSpec
        from jax.experimental.shard_map import shard_map
        from concourse import bass2jax, mybir
        from concourse.bass2jax import (_bass_exec_p, install_neuronx_cc_hook,
                                        partition_id_tensor)

        install_neuronx_cc_hook()
        self.jax = jax
        self.nc = nc
        partition_name = (nc.partition_id_tensor.name
                          if nc.partition_id_tensor else None)

        in_names = []
        out_names = []
        out_avals = []
        zero_shapes = []
        for alloc in nc.m.functions[0].allocations:
            if not isinstance(alloc, mybir.MemoryLocationSet):
                continue
            name = alloc.memorylocations[0].name
            if alloc.kind == "ExternalInput":
                if name != partition_name:
                    in_names.append(name)
            elif alloc.kind == "ExternalOutput":
                shape = tuple(alloc.tensor_shape)
                dtype = mybir.dt.np(alloc.dtype)
                out_names.append(name)
                out_avals.append(jax.core.ShapedArray(shape, dtype))
                zero_shapes.append((shape, dtype))
        self.in_names = list(in_names)
        self.out_names = out_names
        self.out_avals = out_avals
        self.zero_shapes = zero_shapes
        n_params = len(in_names)
        n_outs = len(out_names)
        all_in_names = in_names + out_names
        if partition_name is not None:
            all_in_names = all_in_names + [partition_name]
        self.all_in_names = all_in_names
        self.partition_name = partition_name

        def _body(*args):
            operands = list(args)
            if partition_name is not None:
                operands.append(partition_id_tensor())
            outs = _bass_exec_p.bind(
                *operands,
                out_avals=tuple(out_avals),
                in_names=tuple(all_in_names),
                out_names=tuple(out_names),
                lowering_input_output_aliases=(),
                sim_require_finite=True,
                sim_require_nnan=True,
                nc=nc,
            )
            return tuple(outs)

        devices = jax.devices()[:N_CORES]
        mesh = Mesh(np.asarray(devices), ("core",))
        self.mesh = mesh
        in_specs = (PartitionSpec("core"),) * (n_params + n_outs)
        out_specs = (PartitionSpec("core"),) * n_outs
        donate = tuple(range(n_params, n_params + n_outs))
        self.sharded = jax.jit(
            shard_map(_body, mesh=mesh, in_specs=in_specs,
                      out_specs=out_specs, check_rep=False),
            donate_argnums=donate, keep_unused=True,
        )

    def make_looped(self, n_iters):
        """Jitted fn running the NEFF n_iters times on-device (for timing)."""
        import jax
        import jax.numpy as jnp
        from jax.sharding import PartitionSpec
        from jax.experimental.shard_map import shard_map
        from concourse.bass2jax import _bass_exec_p, partition_id_tensor

        nc = self.nc
        out_avals = self.out_avals
        all_in_names = self.all_in_names
        out_names = self.out_names
        partition_name = self.partition_name

        def _loop(*args):
            def body(i, acc):
                operands = list(args)
                if partition_name is not None:
                    operands.append(partition_id_tensor())
                outs = _bass_exec_p.bind(
                    *operands,
                    out_avals=tuple(out_avals),
                    in_names=tuple(all_in_names),
                    out_names=tuple(out_names),
                    lowering_input_output_aliases=(),
                    sim_require_finite=True,
                    sim_require_nnan=True,
                    nc=nc,
                )
                return acc + outs[0][0, 0].astype(jnp.float32)

            acc = jax.lax.fori_loop(0, n_iters, body, jnp.float32(0.0))
            return (acc.reshape(1),)

        n_in = len(self.in_names) + len(self.out_names)
        return jax.jit(shard_map(
            _loop, mesh=self.mesh,
            in_specs=(PartitionSpec("core"),) * n_in,
            out_specs=(PartitionSpec("core"),),
            check_rep=False,
        ))

    def concat_inputs(self, in_maps):
        return [
            np.concatenate([np.asarray(in_maps[c][n]) for c in range(N_CORES)],
                           axis=0)
            for n in self.in_names
        ]

    def make_zeros(self):
        return [np.zeros((N_CORES * s[0], *s[1:]), d)
                for (s, d) in self.zero_shapes]

    def __call__(self, in_maps):
        concat_in = self.concat_inputs(in_maps)
        out_arrs = self.sharded(*concat_in, *self.make_zeros())
        return [
            {name: np.asarray(out_arrs[i]).reshape(
                N_CORES, *self.out_avals[i].shape)[c]
             for i, name in enumerate(self.out_names)}
            for c in range(N_CORES)
        ]


def get_runner():
    global _RUNNER
    if _RUNNER is None:
        _RUNNER = _Runner(get_graph())
    return _RUNNER


def kernel(x, Wq, Wv, cos, sin):
    x = np.asarray(x, dtype=np.float32)
    Wq = np.asarray(Wq, dtype=np.float32)
    Wv = np.asarray(Wv, dtype=np.float32)
    cos = np.asarray(cos, dtype=np.float32)
    sin = np.asarray(sin, dtype=np.float32)
    in_maps = pack_inputs(x, Wq, Wv, cos, sin)
    results = get_runner()(in_maps)
    return gather_outputs(results)

